# revision 12
# baseline (speedup 1.0000x reference)
"""Trainium2 Bass kernel for GQA attention (B=2, T=2048, C=4096, H=32, KV=8, D=128)
with RoPE and causal mask.

Sharding: tensor-parallel over heads across 8 cores. Each core owns 4 Q heads and
their shared KV head: projects q/k/v for those heads, runs causal attention, and
computes a partial output projection; the host sums the 8 partials.

All on-chip layouts are transposed ([feature, token]) so every matmul consumes
natural slices:
  qT/kT/vT = W^T @ x  via lhsT=W-tile [128c, cols], rhs=xT-tile [128c, 512t]
  sT[tk, tq] = kT-tile^T @ qT-chunk   (per 128-row key tile x 512-col query chunk)
  pT = exp(sT/sqrt(D) - 10) on ACT; strictly-causal-upper tiles skipped entirely
  yT[d, tq] += v-tile^T @ pT          (v pre-transposed to [t, d] via PE transpose)
  out[tq, :] += yT_h^T @ wo_h         (accumulate 4 heads in PSUM, DMA straight out)
Softmax denominator: partition-halving adds on DVE; normalization multiplies yT by
a PE outer-product broadcast of the reciprocal row.
"""

import os
from collections import deque
from contextlib import ExitStack

import numpy as np
import ml_dtypes

import concourse.bass as bass
import concourse.bacc as bacc
import concourse.mybir as mybir
import concourse.tile as tile

BF = mybir.dt.bfloat16
F32 = mybir.dt.float32
F32R = mybir.dt.float32r
AFT = mybir.ActivationFunctionType

NCORES = 8
B, T, C = 2, 2048, 4096
H, KV, D = 32, 8, 128
QH = H // NCORES          # 4 q-heads per core
CT = C // 128             # 32 contraction tiles
NCH = T // 512            # 4 query chunks per batch
SCALE = 1.0 / float(np.sqrt(D))
EXP_BIAS = -10.0
ROPE_BASE = 10000.0

bf16 = ml_dtypes.bfloat16


def emit_program():
    nc = bacc.Bacc("TRN2", target_bir_lowering=False, debug=False,
                   num_devices=NCORES)

    xT_d = nc.dram_tensor("xT", [C, B * T], BF, kind="ExternalInput").ap()
    wq_d = nc.dram_tensor("wq", [C, QH * D], BF, kind="ExternalInput").ap()
    wk_d = nc.dram_tensor("wk", [C, D], BF, kind="ExternalInput").ap()
    wv_d = nc.dram_tensor("wv", [C, D], BF, kind="ExternalInput").ap()
    wo_d = nc.dram_tensor("woA", [128, QH, C], BF, kind="ExternalInput").ap()
    cos_d = nc.dram_tensor("cosT", [D, T], BF, kind="ExternalInput").ap()
    sin_d = nc.dram_tensor("sinTr", [D, T], BF, kind="ExternalInput").ap()
    alw_d = nc.dram_tensor("allowA", [128, 4, 512], BF, kind="ExternalInput").ap()
    id_d = nc.dram_tensor("ident", [128, 128], BF, kind="ExternalInput").ap()
    out_d = nc.dram_tensor("out", [B * T, C], F32, kind="ExternalOutput").ap()

    with tile.TileContext(nc) as tc, ExitStack() as ctx:
        const = ctx.enter_context(tc.tile_pool(name="const", bufs=1))
        act = ctx.enter_context(tc.tile_pool(name="act", bufs=1))
        work = ctx.enter_context(tc.tile_pool(name="work", bufs=1))

        wq_sb = const.tile([128, CT, QH * D], BF)
        nc.sync.dma_start(wq_sb[:], wq_d.rearrange("(ci p) n -> p ci n", p=128))
        wk_sb = const.tile([128, CT, D], BF)
        nc.sync.dma_start(wk_sb[:], wk_d.rearrange("(ci p) n -> p ci n", p=128))
        wv_sb = const.tile([128, CT, D], BF)
        nc.sync.dma_start(wv_sb[:], wv_d.rearrange("(ci p) n -> p ci n", p=128))
        wo_sb = const.tile([128, QH, C], BF)
        nc.sync.dma_start(wo_sb[:], wo_d)
        cos_sb = const.tile([D, T], BF)
        nc.sync.dma_start(cos_sb[:], cos_d)
        sin_sb = const.tile([D, T], BF)
        nc.sync.dma_start(sin_sb[:], sin_d)
        alw_sb = const.tile([128, 4, 512], BF)
        nc.sync.dma_start(alw_sb[:], alw_d)
        id_sb = const.tile([128, 128], BF)
        nc.sync.dma_start(id_sb[:], id_d)
        ones_sb = const.tile([1, 128], F32)
        nc.gpsimd.memset(ones_sb[:], 1.0)
        onesbf_sb = const.tile([128, 1], BF)
        nc.gpsimd.memset(onesbf_sb[:], 1.0)
        bias_sb = const.tile([128, 1], F32)
        nc.gpsimd.memset(bias_sb[:], EXP_BIAS)

        def rope_evict(dst, psum, cs):
            # dst = psum * cos + swap_halves(psum) * sin_rot   (bf16 out)
            sw = work.tile([128, 512], F32, tag="sw", bufs=3, name="sw")
            nc.scalar.copy(sw[0:64, :], psum[64:128, :])
            nc.scalar.copy(sw[64:128, :], psum[0:64, :])
            nc.vector.tensor_mul(sw[:], sw[:], sin_sb[:, cs])
            cst = work.tile([128, 512], F32, tag="cst", bufs=3, name="cst")
            nc.vector.tensor_mul(cst[:], psum[:], cos_sb[:, cs])
            nc.vector.tensor_add(dst, cst[:], sw[:])

        for b in range(B):
            qT = act.tile([D, QH, T], BF, tag="qT", name="qT")
            kT = act.tile([D, T], BF, tag="kT", name="kT")
            vT = act.tile([D, T], BF, tag="vT", name="vT")
            vsb = act.tile([128, T // 128, D], BF, tag="v", name="vsb")

            # ---- projections ----
            with tc.tile_pool(name=f"pproj{b}", bufs=1, space="PSUM") as pp:
                for jc in range(NCH):
                    pq = [pp.tile([128, 512], F32, tag=f"pq{h}", name=f"pq{h}")
                          for h in range(QH)]
                    pk = pp.tile([128, 512], F32, tag="pk", name="pk")
                    pv = pp.tile([128, 512], F32, tag="pv", name="pv")
                    for ci in range(CT):
                        xt = work.tile([128, 512], BF, tag="xt", bufs=6, name="xt")
                        col0 = b * T + 512 * jc
                        nc.sync.dma_start(
                            xt[:], xT_d[128 * ci:128 * (ci + 1), col0:col0 + 512])
                        st, sp = ci == 0, ci == CT - 1
                        for h in range(QH):
                            nc.tensor.matmul(
                                pq[h][:], wq_sb[:, ci, 128 * h:128 * (h + 1)],
                                xt[:], start=st, stop=sp)
                        nc.tensor.matmul(pk[:], wk_sb[:, ci, :], xt[:],
                                         start=st, stop=sp)
                        nc.tensor.matmul(pv[:], wv_sb[:, ci, :], xt[:],
                                         start=st, stop=sp)
                    cs = slice(512 * jc, 512 * (jc + 1))
                    for h in range(QH):
                        rope_evict(qT[:, h, cs], pq[h][:], cs)
                    rope_evict(kT[:, cs], pk[:], cs)
                    nc.scalar.copy(vT[:, cs], pv[:])

            # ---- transpose v to [t, d] tiles ----
            with tc.tile_pool(name=f"ptr{b}", bufs=2, space="PSUM") as ptr:
                for k in range(T // 128):
                    tp = ptr.tile([128, 128], BF, tag="tp", name="tp")
                    nc.tensor.transpose(tp[:], vT[:, 128 * k:128 * (k + 1)],
                                        id_sb[:])
                    nc.vector.tensor_copy(vsb[:, k, :], tp[:])

            # ---- attention + output projection ----
            with tc.tile_pool(name=f"pattn{b}", bufs=1, space="PSUM") as pa:
                wo_jobs = deque()

                def make_wo_job(b, j, tl, o, yts):
                    def job():
                        ops = pa.tile([128, 512], F32, tag="ops", bufs=2,
                                      name="ops")
                        for h in range(QH):
                            nc.tensor.matmul(
                                ops[:], yts[h][:, 128 * tl:128 * (tl + 1)],
                                wo_sb[:, h, 512 * o:512 * (o + 1)],
                                start=h == 0, stop=h == QH - 1)
                        ob = work.tile([128, 512], F32, tag="ob", bufs=4,
                                       name="ob")
                        nc.any.tensor_copy(ob[:], ops[:])
                        r0 = b * T + 512 * j + 128 * tl
                        nc.sync.dma_start(out_d[r0:r0 + 128, 512 * o:512 * (o + 1)],
                                          ob[:])
                    return job

                for j in range(NCH):
                    yts = {}
                    for h in range(QH):
                        yps = pa.tile([128, 512], F32, tag="yps", bufs=2,
                                      name="yps")
                        dps = pa.tile([1, 512], F32, tag="dps", bufs=2,
                                      name="dps")
                        K = 4 * j + 4
                        for k in range(K):
                            sps = pa.tile([128, 512], F32, tag="sps", bufs=2,
                                          name="sps")
                            nc.tensor.matmul(
                                sps[:], kT[:, 128 * k:128 * (k + 1)],
                                qT[:, h, 512 * j:512 * (j + 1)],
                                start=True, stop=True)
                            pt = work.tile([128, 512], BF, tag="pt", bufs=4,
                                           name="pt")
                            nc.scalar.activation(pt[:], sps[:], AFT.Exp,
                                                 bias=bias_sb[:], scale=SCALE)
                            o = k - 4 * j
                            if o >= 0:
                                nc.vector.tensor_mul(pt[:], pt[:],
                                                     alw_sb[:, o, :])
                            nc.tensor.matmul(dps[:], onesbf_sb[:], pt[:],
                                             start=(k == 0), stop=(k == K - 1))
                            nc.tensor.matmul(yps[:], vsb[:, k, :], pt[:],
                                             start=(k == 0), stop=(k == K - 1))
                            if wo_jobs:
                                wo_jobs.popleft()()
                        rec = work.tile([1, 512], F32, tag="rec", bufs=2,
                                        name="rec")
                        nc.vector.reciprocal(rec[:], dps[:])
                        bc = pa.tile([128, 512], F32, tag="sps", bufs=2,
                                     name="bc")
                        nc.tensor.matmul(bc[:], ones_sb[:], rec[:],
                                         start=True, stop=True)
                        bcs = work.tile([128, 512], F32, tag="bcs", bufs=2,
                                        name="bcs")
                        nc.any.tensor_copy(bcs[:], bc[:])
                        yt = work.tile([128, 512], BF, tag="yt", bufs=8,
                                       name="yt")
                        nc.vector.tensor_mul(yt[:], yps[:], bcs[:])
                        yts[h] = yt
                    for tl in range(4):
                        for o in range(C // 512):
                            wo_jobs.append(make_wo_job(b, j, tl, o, yts))
                while wo_jobs:
                    wo_jobs.popleft()()

    nc.compile()
    return nc


def host_prep(inputs):
    x = np.asarray(inputs["x"], np.float32)
    mask = np.asarray(inputs["mask"], np.float32)
    wq = np.asarray(inputs["wq"], np.float32)
    wk = np.asarray(inputs["wk"], np.float32)
    wv = np.asarray(inputs["wv"], np.float32)
    wo = np.asarray(inputs["wo"], np.float32)

    xT = np.ascontiguousarray(x.reshape(B * T, C).T).astype(bf16)
    inv = 1.0 / (ROPE_BASE ** (np.arange(0, D, 2, dtype=np.float64) / D))
    freqs = np.arange(T, dtype=np.float64)[:, None] * inv[None, :] * B
    emb = np.concatenate([freqs, freqs], axis=-1)       # [T, D]
    cosT = np.cos(emb).T.astype(np.float32).astype(bf16)
    sinT = np.sin(emb).T.astype(np.float32)
    sinT[: D // 2] *= -1.0
    sinTr = sinT.astype(bf16)
    # allow[p, o, jj] = 1 - mask[jj, 128*o + p]  (from the actual mask input)
    allowA = np.ascontiguousarray(
        np.stack([(1.0 - mask[0:512, 128 * o:128 * (o + 1)]).T
                  for o in range(4)], axis=1)).astype(bf16)   # [128, 4, 512]
    ident = np.eye(128, dtype=np.float32).astype(bf16)

    common = dict(xT=xT, cosT=cosT, sinTr=sinTr, allowA=allowA, ident=ident)
    in_maps = []
    for c in range(NCORES):
        m = dict(common)
        m["wq"] = np.ascontiguousarray(wq[:, 512 * c:512 * (c + 1)]).astype(bf16)
        m["wk"] = np.ascontiguousarray(wk[:, 128 * c:128 * (c + 1)]).astype(bf16)
        m["wv"] = np.ascontiguousarray(wv[:, 128 * c:128 * (c + 1)]).astype(bf16)
        m["woA"] = np.ascontiguousarray(
            wo[512 * c:512 * (c + 1), :].reshape(QH, 128, C)
            .transpose(1, 0, 2)).astype(bf16)
        in_maps.append(m)
    return in_maps


def kernel(**inputs) -> np.ndarray:
    from concourse.bass_utils import run_bass_kernel_spmd

    in_maps = host_prep(inputs)
    nc = emit_program()
    trace = bool(os.environ.get("BASS_KERNEL_TRACE"))
    res = run_bass_kernel_spmd(nc, in_maps, core_ids=list(range(NCORES)),
                               trace=trace)
    if trace and res.exec_time_ns is not None:
        print(f"HW exec time: {res.exec_time_ns} ns")
        if res.instructions_and_trace is not None:
            print("trace:", res.instructions_and_trace[1])
    total = np.zeros((B * T, C), np.float32)
    for r in res.results:
        total += r["out"]
    return total.reshape(B, T, C)


# revision 18
# speedup vs baseline: 1.0207x; 1.0207x over previous
"""Trainium2 Bass kernel for GQA attention (B=2, T=2048, C=4096, H=32, KV=8, D=128)
with RoPE and causal mask.

Sharding: tensor-parallel over heads across 8 cores. Each core owns 4 Q heads and
their shared KV head: projects q/k/v for those heads, runs causal attention, and
computes a partial output projection; the host sums the 8 partials.

All on-chip layouts are transposed ([feature, token]) so every matmul consumes
natural slices:
  qT/kT/vT = W^T @ x  via lhsT=W-tile [128c, cols], rhs=xT-tile [128c, 512t]
  sT[tk, tq] = kT-tile^T @ qT-chunk   (per 128-row key tile x 512-col query chunk)
  pT = exp(sT/sqrt(D) - 10) on ACT; strictly-causal-upper tiles skipped entirely
  yT[d, tq] += v-tile^T @ pT          (v pre-transposed to [t, d] via PE transpose)
  out[tq, :] += yT_h^T @ wo_h         (accumulate 4 heads in PSUM, DMA straight out)
Softmax denominator: partition-halving adds on DVE; normalization multiplies yT by
a PE outer-product broadcast of the reciprocal row.
"""

import os
from collections import deque
from contextlib import ExitStack

import numpy as np
import ml_dtypes

import concourse.bass as bass
import concourse.bacc as bacc
import concourse.mybir as mybir
import concourse.tile as tile

BF = mybir.dt.bfloat16
F32 = mybir.dt.float32
F32R = mybir.dt.float32r
AFT = mybir.ActivationFunctionType

NCORES = 8
B, T, C = 2, 2048, 4096
H, KV, D = 32, 8, 128
QH = H // NCORES          # 4 q-heads per core
CT = C // 128             # 32 contraction tiles
NCH = T // 512            # 4 query chunks per batch
SCALE = 1.0 / float(np.sqrt(D))
EXP_BIAS = -10.0
ROPE_BASE = 10000.0

bf16 = ml_dtypes.bfloat16


def emit_program():
    nc = bacc.Bacc("TRN2", target_bir_lowering=False, debug=False,
                   num_devices=NCORES)

    xT_d = nc.dram_tensor("xT", [C, B * T], BF, kind="ExternalInput").ap()
    wq_d = nc.dram_tensor("wq", [C, QH * D], BF, kind="ExternalInput").ap()
    wk_d = nc.dram_tensor("wk", [C, D], BF, kind="ExternalInput").ap()
    wv_d = nc.dram_tensor("wv", [C, D], BF, kind="ExternalInput").ap()
    wo_d = nc.dram_tensor("woA", [128, QH, C], BF, kind="ExternalInput").ap()
    cos_d = nc.dram_tensor("cosT", [D, T], BF, kind="ExternalInput").ap()
    sin_d = nc.dram_tensor("sinTr", [D, T], BF, kind="ExternalInput").ap()
    alw_d = nc.dram_tensor("allowA", [128, 4, 512], BF, kind="ExternalInput").ap()
    id_d = nc.dram_tensor("ident", [128, 128], BF, kind="ExternalInput").ap()
    onesr_d = nc.dram_tensor("onesr", [1, 128], F32R, kind="ExternalInput").ap()
    out_d = nc.dram_tensor("out", [B * T, C], F32, kind="ExternalOutput").ap()

    with tile.TileContext(nc) as tc, ExitStack() as ctx:
        const = ctx.enter_context(tc.tile_pool(name="const", bufs=1))
        act = ctx.enter_context(tc.tile_pool(name="act", bufs=1))
        work = ctx.enter_context(tc.tile_pool(name="work", bufs=1))

        wq_sb = const.tile([128, CT, QH * D], BF)
        nc.sync.dma_start(wq_sb[:], wq_d.rearrange("(ci p) n -> p ci n", p=128))
        wk_sb = const.tile([128, CT, D], BF)
        nc.sync.dma_start(wk_sb[:], wk_d.rearrange("(ci p) n -> p ci n", p=128))
        wv_sb = const.tile([128, CT, D], BF)
        nc.sync.dma_start(wv_sb[:], wv_d.rearrange("(ci p) n -> p ci n", p=128))
        wo_sb = const.tile([128, QH, C], BF)
        nc.sync.dma_start(wo_sb[:], wo_d)
        cos_sb = const.tile([D, T], BF)
        nc.sync.dma_start(cos_sb[:], cos_d)
        sin_sb = const.tile([D, T], BF)
        nc.sync.dma_start(sin_sb[:], sin_d)
        alw_sb = const.tile([128, 4, 512], BF)
        nc.sync.dma_start(alw_sb[:], alw_d)
        id_sb = const.tile([128, 128], BF)
        nc.sync.dma_start(id_sb[:], id_d)
        ones_sb = const.tile([1, 128], F32R)
        nc.sync.dma_start(ones_sb[:], onesr_d)
        onesbf_sb = const.tile([128, 1], BF)
        nc.gpsimd.memset(onesbf_sb[:], 1.0)
        bias_sb = const.tile([128, 1], F32)
        nc.gpsimd.memset(bias_sb[:], EXP_BIAS)

        def rope_evict(dst, psum, cs):
            # dst = psum * cos + swap_halves(psum) * sin_rot   (bf16 out)
            sw = work.tile([128, 512], F32, tag="sw", bufs=3, name="sw")
            nc.scalar.copy(sw[0:64, :], psum[64:128, :])
            nc.scalar.copy(sw[64:128, :], psum[0:64, :])
            nc.vector.tensor_mul(sw[:], sw[:], sin_sb[:, cs])
            cst = work.tile([128, 512], F32, tag="cst", bufs=3, name="cst")
            nc.vector.tensor_mul(cst[:], psum[:], cos_sb[:, cs])
            nc.vector.tensor_add(dst, cst[:], sw[:])

        for b in range(B):
            qT = act.tile([D, QH, T], BF, tag="qT", name="qT")
            kT = act.tile([D, T], BF, tag="kT", name="kT")
            vT = act.tile([D, T], BF, tag="vT", name="vT")
            vsb = act.tile([128, T // 128, D], BF, tag="v", name="vsb")

            # ---- projections ----
            with tc.tile_pool(name=f"pproj{b}", bufs=1, space="PSUM") as pp:
                for jc in range(NCH):
                    pq = [pp.tile([128, 512], F32, tag=f"pq{h}", name=f"pq{h}")
                          for h in range(QH)]
                    pk = pp.tile([128, 512], F32, tag="pk", name="pk")
                    pv = pp.tile([128, 512], F32, tag="pv", name="pv")
                    for ci in range(CT):
                        xt = work.tile([128, 512], BF, tag="xt", bufs=6, name="xt")
                        col0 = b * T + 512 * jc
                        nc.sync.dma_start(
                            xt[:], xT_d[128 * ci:128 * (ci + 1), col0:col0 + 512])
                        st, sp = ci == 0, ci == CT - 1
                        for h in range(QH):
                            nc.tensor.matmul(
                                pq[h][:], wq_sb[:, ci, 128 * h:128 * (h + 1)],
                                xt[:], start=st, stop=sp)
                        nc.tensor.matmul(pk[:], wk_sb[:, ci, :], xt[:],
                                         start=st, stop=sp)
                        nc.tensor.matmul(pv[:], wv_sb[:, ci, :], xt[:],
                                         start=st, stop=sp)
                    cs = slice(512 * jc, 512 * (jc + 1))
                    for h in range(QH):
                        rope_evict(qT[:, h, cs], pq[h][:], cs)
                    rope_evict(kT[:, cs], pk[:], cs)
                    nc.scalar.copy(vT[:, cs], pv[:])

            # ---- transpose v to [t, d] tiles ----
            with tc.tile_pool(name=f"ptr{b}", bufs=2, space="PSUM") as ptr:
                for k in range(T // 128):
                    tp = ptr.tile([128, 128], BF, tag="tp", name="tp")
                    nc.tensor.transpose(tp[:], vT[:, 128 * k:128 * (k + 1)],
                                        id_sb[:])
                    nc.vector.tensor_copy(vsb[:, k, :], tp[:])

            # ---- attention + output projection ----
            with tc.tile_pool(name=f"pattn{b}", bufs=1, space="PSUM") as pa:
                wo_jobs = deque()

                def make_wo_job(b, j, tl, o, yts):
                    def job():
                        ops = pa.tile([128, 512], F32, tag="ops", bufs=1,
                                      name="ops")
                        for h in range(QH):
                            nc.tensor.matmul(
                                ops[:], yts[h][:, 128 * tl:128 * (tl + 1)],
                                wo_sb[:, h, 512 * o:512 * (o + 1)],
                                start=h == 0, stop=h == QH - 1)
                        ob = work.tile([128, 512], F32, tag="ob", bufs=4,
                                       name="ob")
                        nc.vector.tensor_copy(ob[:], ops[:])
                        r0 = b * T + 512 * j + 128 * tl
                        nc.sync.dma_start(out_d[r0:r0 + 128, 512 * o:512 * (o + 1)],
                                          ob[:])
                    return job

                def make_fin_job(yps, dps, yt):
                    # softmax normalization tail for one head, deferred so the
                    # in-order PE queue never stalls on the reciprocal
                    def job():
                        rec = work.tile([1, 512], F32R, tag="rec", bufs=2,
                                        name="rec")
                        with nc.allow_low_precision(reason="f32r recip bcast"):
                            nc.vector.reciprocal(rec[:], dps[:])
                        bc = pa.tile([128, 512], F32, tag="bc", bufs=1,
                                     name="bc")
                        nc.tensor.matmul(bc[:], ones_sb[:], rec[:],
                                         start=True, stop=True)
                        bcs = work.tile([128, 512], F32, tag="bcs", bufs=2,
                                        name="bcs")
                        nc.vector.tensor_copy(bcs[:], bc[:])
                        nc.vector.tensor_mul(yt[:], yps[:], bcs[:])
                    return job

                for j in range(NCH):
                    yts = {}
                    for h in range(QH):
                        yps = pa.tile([128, 512], F32, tag="yps", bufs=2,
                                      name="yps")
                        dps = pa.tile([1, 512], F32, tag="dps", bufs=2,
                                      name="dps")
                        K = 4 * j + 4
                        for k in range(K):
                            sps = pa.tile([128, 512], F32, tag="sps", bufs=2,
                                          name="sps")
                            nc.tensor.matmul(
                                sps[:], kT[:, 128 * k:128 * (k + 1)],
                                qT[:, h, 512 * j:512 * (j + 1)],
                                start=True, stop=True)
                            pt = work.tile([128, 512], BF, tag="pt", bufs=4,
                                           name="pt")
                            nc.scalar.activation(pt[:], sps[:], AFT.Exp,
                                                 bias=bias_sb[:], scale=SCALE)
                            o = k - 4 * j
                            if o >= 0:
                                nc.vector.tensor_mul(pt[:], pt[:],
                                                     alw_sb[:, o, :])
                            nc.tensor.matmul(dps[:], onesbf_sb[:], pt[:],
                                             start=(k == 0), stop=(k == K - 1))
                            nc.tensor.matmul(yps[:], vsb[:, k, :], pt[:],
                                             start=(k == 0), stop=(k == K - 1))
                            if wo_jobs:
                                wo_jobs.popleft()()
                        yt = work.tile([128, 512], BF, tag="yt", bufs=8,
                                       name="yt")
                        wo_jobs.append(make_fin_job(yps, dps, yt))
                        yts[h] = yt
                    for tl in range(4):
                        for o in range(C // 512):
                            wo_jobs.append(make_wo_job(b, j, tl, o, yts))
                while wo_jobs:
                    wo_jobs.popleft()()

    nc.compile()
    return nc


def host_prep(inputs):
    x = np.asarray(inputs["x"], np.float32)
    mask = np.asarray(inputs["mask"], np.float32)
    wq = np.asarray(inputs["wq"], np.float32)
    wk = np.asarray(inputs["wk"], np.float32)
    wv = np.asarray(inputs["wv"], np.float32)
    wo = np.asarray(inputs["wo"], np.float32)

    xT = np.ascontiguousarray(x.reshape(B * T, C).T).astype(bf16)
    inv = 1.0 / (ROPE_BASE ** (np.arange(0, D, 2, dtype=np.float64) / D))
    freqs = np.arange(T, dtype=np.float64)[:, None] * inv[None, :] * B
    emb = np.concatenate([freqs, freqs], axis=-1)       # [T, D]
    cosT = np.cos(emb).T.astype(np.float32).astype(bf16)
    sinT = np.sin(emb).T.astype(np.float32)
    sinT[: D // 2] *= -1.0
    sinTr = sinT.astype(bf16)
    # allow[p, o, jj] = 1 - mask[jj, 128*o + p]  (from the actual mask input)
    allowA = np.ascontiguousarray(
        np.stack([(1.0 - mask[0:512, 128 * o:128 * (o + 1)]).T
                  for o in range(4)], axis=1)).astype(bf16)   # [128, 4, 512]
    ident = np.eye(128, dtype=np.float32).astype(bf16)

    common = dict(xT=xT, cosT=cosT, sinTr=sinTr, allowA=allowA, ident=ident,
                  onesr=np.ones((1, 128), np.float32))
    in_maps = []
    for c in range(NCORES):
        m = dict(common)
        m["wq"] = np.ascontiguousarray(wq[:, 512 * c:512 * (c + 1)]).astype(bf16)
        m["wk"] = np.ascontiguousarray(wk[:, 128 * c:128 * (c + 1)]).astype(bf16)
        m["wv"] = np.ascontiguousarray(wv[:, 128 * c:128 * (c + 1)]).astype(bf16)
        m["woA"] = np.ascontiguousarray(
            wo[512 * c:512 * (c + 1), :].reshape(QH, 128, C)
            .transpose(1, 0, 2)).astype(bf16)
        in_maps.append(m)
    return in_maps


def kernel(**inputs) -> np.ndarray:
    from concourse.bass_utils import run_bass_kernel_spmd

    in_maps = host_prep(inputs)
    nc = emit_program()
    trace = bool(os.environ.get("BASS_KERNEL_TRACE"))
    res = run_bass_kernel_spmd(nc, in_maps, core_ids=list(range(NCORES)),
                               trace=trace)
    if trace and res.exec_time_ns is not None:
        print(f"HW exec time: {res.exec_time_ns} ns")
        if res.instructions_and_trace is not None:
            print("trace:", res.instructions_and_trace[1])
    total = np.zeros((B * T, C), np.float32)
    for r in res.results:
        total += r["out"]
    return total.reshape(B, T, C)
